# revision 37
# baseline (speedup 1.0000x reference)
"""Trainium2 Bass kernel for nn_DeformConv2D_EPF (scatter_memory).

Contract: kernel(**inputs) takes the FULL unsharded inputs
  x [512, 202, 27, 27] f32, W [64, 200, 3, 3] f32, b [64] f32,
  rand_idx [512, 58] int32
and returns the FULL output y [512, 64, 27, 27] f32.

Strategy (pure data parallel, batch 512 -> 8 cores x 64 samples), all
data-dependent work on device.

Key idea: per sample, every pixel of the deformed conv operand is one of
the <=128 "compact" rows (the central-superpixel pixels), i.e. the
operand has rank <= 128.  The whole scatter/gather is therefore
expressed as one-hot matmuls on the PE instead of SWDGE dma_gather
descriptor storms (the previous bottleneck at ~10us/sample):

  hc[c,r]    = sum_n hyper[n,c] * Q[n,r]     Q[n,r] = (rho[n] == r)
  UT[r,t,o]  = sum_c hc[c,r] * W[o,c,t]      per-tap weighted compact rows
  y[o,n]     = sum_t sum_r UT[r,t,o] * P[r, pad(n)+off(t)]
               P[r,j] = (ridx2[j] == r)      3x3 conv, contraction over r

Q and P are one-hots built by DVE is_equal against iota constants:
  - rho (compact rank per pixel, -1 if none) is computed in phase A
    ([samples, pixels] layout), round-trips through DRAM and comes back
    pixel-major via 6 xbar transpose DMAs -> per-(chunk,sample) scalar
    columns for Q.
  - ridx2 (source compact row per padded position, via zrank mod m) is
    scattered into padded layout by GPSIMD local_scatter, then
    partition-replicated per sample by one SBUF->SBUF broadcast DMA.

Conv matmuls are column-tiled in sample pairs: sample A's 18 taps write
PSUM partitions 0:64 (PE columns 0:63), sample B's write 64:128 via
tile_position (0,64), so two samples stream the 128-wide PE array
concurrently.  PE work/sample ~7k cycles; everything else overlaps.
"""

import os
from contextlib import ExitStack

import ml_dtypes
import numpy as np

B, C, P = 512, 202, 27
CH, N = 200, P * P            # 200, 729
PP = 29                       # padded side
NPAD = PP * PP                # 841
JW = 896                      # padded-position row width (841 padded)
NF = 736                      # pixel count padded for phase A
OUTC = 64
S = int(os.environ.get("KERNEL_S", "64"))   # samples per core (env: sim tests)
N_CORES = 8
CENTER = (P // 2) * P + P // 2  # 364
NK = 6                        # pixel chunks of 128 (768 padded)

_BF16 = ml_dtypes.bfloat16


def _stage_core(x, W, b, rand_idx):
    """Input map for one core's sample slice. Layout/dtype reformat only."""
    ns = x.shape[0]
    hyper = np.ascontiguousarray(
        x[:, :CH].reshape(ns, CH, N).transpose(0, 2, 1))     # [ns, N, CH]
    hyp = np.zeros((ns, NK * 128, 256), dtype=_BF16)
    hyp[:, :N, :CH] = hyper.astype(_BF16)
    # on-chip layout: partition = n%128, free = (k, c)
    hyp = np.ascontiguousarray(
        hyp.reshape(ns, NK, 128, 256).transpose(0, 2, 1, 3)
        .reshape(ns, 128, NK * 256))

    spr = np.zeros((ns, NF), np.float32)
    spr[:, :N] = x[:, CH].reshape(ns, N)

    rnd = np.full((ns, 64), -1, np.int16)
    rnd[:, :58] = rand_idx.astype(np.int16)

    wr = np.zeros((128, 1152), dtype=_BF16)
    Wt = W.transpose(1, 2, 3, 0).reshape(CH, 9, OUTC)         # [c, t, o]
    wr[:, :576] = Wt[:128].reshape(128, 576).astype(_BF16)
    wr[: CH - 128, 576:] = Wt[128:].reshape(CH - 128, 576).astype(_BF16)

    bias2 = np.tile(b.astype(np.float32), 2).reshape(128, 1)

    iota_f = np.broadcast_to(np.arange(NF, dtype=np.float32), (ns, NF)).copy()
    iota1_i = np.broadcast_to(
        (np.arange(NF) + 1).astype(np.int16), (ns, NF)).copy()
    n = np.arange(N)
    flatpad = (n // P + 1) * PP + (n % P + 1)
    padf0 = np.full(NF, -1, np.int16)
    padf0[:N] = flatpad
    padf = np.broadcast_to(padf0, (ns, NF)).copy()
    # iota along r, tiled for a whole (pair, chunk) Q build in one DVE op
    iotaR = np.broadcast_to(
        np.arange(128, dtype=np.float32), (128, 2 * NK * 128)
        if False else (128, 2, NK, 128)).reshape(128, 2 * NK * 128)
    iotaR = np.ascontiguousarray(iotaR).astype(_BF16)
    iotaC1 = (np.arange(128, dtype=np.float32) + 1).reshape(128, 1)
    ones = np.ones((1, 128), dtype=_BF16)

    return {
        "hyp": hyp, "spr": spr, "rnd": rnd, "wr": wr, "bias2": bias2,
        "c_iota_f": iota_f, "c_iota1": iota1_i, "c_padf": padf,
        "c_iotaR": iotaR, "c_iotaC1": iotaC1, "c_ones": ones,
    }


def _host_stage(x, W, b, rand_idx):
    in_maps = []
    for c in range(N_CORES):
        sl = slice(c * S, (c + 1) * S)
        in_maps.append(_stage_core(x[sl], W, b, rand_idx[sl]))
    return in_maps


# ---------------------------------------------------------------- bass build

def build_nc(loop_n=1, mut=None):
    mut = mut or os.environ.get("KERNEL_MUT", "")
    import concourse.bass as bass
    import concourse.tile as tile
    from concourse import bacc, mybir, library_config
    from concourse.tile_rust import add_dep_helper

    dt = mybir.dt
    Alu = mybir.AluOpType

    nc = bacc.Bacc("TRN2", target_bir_lowering=False, debug=False,
                   num_devices=N_CORES, num_swdge_queues=1)

    hyp = nc.dram_tensor("hyp", [S, 128, NK * 256], dt.bfloat16,
                         kind="ExternalInput").ap()
    spr = nc.dram_tensor("spr", [S, NF], dt.float32, kind="ExternalInput").ap()
    rnd = nc.dram_tensor("rnd", [S, 64], dt.int16, kind="ExternalInput").ap()
    wr = nc.dram_tensor("wr", [128, 1152], dt.bfloat16,
                        kind="ExternalInput").ap()
    bias2 = nc.dram_tensor("bias2", [128, 1], dt.float32,
                           kind="ExternalInput").ap()
    c_iota_f = nc.dram_tensor("c_iota_f", [S, NF], dt.float32,
                              kind="ExternalInput").ap()
    c_iota1 = nc.dram_tensor("c_iota1", [S, NF], dt.int16,
                             kind="ExternalInput").ap()
    c_padf = nc.dram_tensor("c_padf", [S, NF], dt.int16,
                            kind="ExternalInput").ap()
    c_iotaR = nc.dram_tensor("c_iotaR", [128, 2 * NK * 128], dt.bfloat16,
                             kind="ExternalInput").ap()
    c_iotaC1 = nc.dram_tensor("c_iotaC1", [128, 1], dt.float32,
                              kind="ExternalInput").ap()
    c_ones = nc.dram_tensor("c_ones", [1, 128], dt.bfloat16,
                            kind="ExternalInput").ap()
    y = nc.dram_tensor("y", [S, OUTC, N], dt.float32,
                       kind="ExternalOutput").ap()
    rho_d = nc.dram_tensor("rho_d", [S, NK * 128], dt.bfloat16,
                           kind="Internal").ap()

    with tile.TileContext(nc) as tc, ExitStack() as ctx:
        persist = ctx.enter_context(tc.tile_pool(name="persist", bufs=1))
        hpool = ctx.enter_context(tc.tile_pool(name="hpool", bufs=2))
        rppool = ctx.enter_context(tc.tile_pool(name="rppool", bufs=2))
        qpool = ctx.enter_context(tc.tile_pool(name="qpool", bufs=2))
        hcpool = ctx.enter_context(tc.tile_pool(name="hcpool", bufs=2))
        utpool = ctx.enter_context(tc.tile_pool(name="utpool", bufs=3))
        ppool = ctx.enter_context(tc.tile_pool(name="ppool", bufs=3))
        ybpool = ctx.enter_context(tc.tile_pool(name="ybpool", bufs=2))
        psA = ctx.enter_context(tc.tile_pool(name="psA", bufs=2, space="PSUM"))
        psU = ctx.enter_context(tc.tile_pool(name="psU", bufs=1, space="PSUM"))
        psP = ctx.enter_context(tc.tile_pool(name="psP", bufs=1, space="PSUM"))
        psY = ctx.enter_context(tc.tile_pool(name="psY", bufs=1, space="PSUM"))

        V = nc.vector
        A = nc.scalar
        GP = nc.gpsimd
        f32 = dt.float32

        def body():
            # ---------------- persistent loads
            def load(name, src, shape, dty):
                t = persist.tile(shape, dty, tag=name)
                nc.sync.dma_start(t[:], src)
                return t

            sp_t = load("sp", spr, [S, NF], f32)
            rnd_t = load("rnd", rnd, [S, 64], dt.int16)
            wr_t = load("wr", wr, [128, 1152], dt.bfloat16)
            bias_t = load("bias2", bias2, [128, 1], f32)
            iof_t = load("iof", c_iota_f, [S, NF], f32)
            io1_t = load("io1", c_iota1, [S, NF], dt.int16)
            padf_t = load("padf", c_padf, [S, NF], dt.int16)
            iR_t = load("iR", c_iotaR, [128, 2 * NK * 128], dt.bfloat16)
            iC1_t = load("iC1", c_iotaC1, [128, 1], f32)
            ones_t = load("ones", c_ones, [1, 128], dt.bfloat16)

            GP.load_library(library_config.local_scatter)

            # ---------------- phase A: index math (samples on partitions)
            central = persist.tile([S, 1], f32, tag="central")
            V.tensor_copy(central[:], sp_t[:, CENTER:CENTER + 1])

            mask = persist.tile([S, NF], f32, tag="mask")
            V.tensor_scalar(mask[:], sp_t[:], central[:], None, Alu.is_equal)
            # rand one-hot via per-partition scatter of positive data
            rsc = persist.tile([S, NF], dt.int16, tag="rsc")
            rscf = persist.tile([S, NF], f32, tag="rscf")
            GP.local_scatter(rsc[:], io1_t[:, 0:64], rnd_t[:],
                             channels=S, num_elems=NF, num_idxs=64)
            V.tensor_copy(rscf[:], rsc[:])
            V.tensor_scalar(rscf[:], rscf[:], 0.0, None, Alu.is_gt)
            V.tensor_tensor(mask[:], mask[:], rscf[:], Alu.max)

            oscan = persist.tile([S, NF], f32, tag="oscan")
            V.tensor_tensor_scan(oscan[:], mask[:], mask[:], 0.0,
                                 Alu.add, Alu.bypass)
            m_col = oscan[:, NF - 1:NF]                 # ones count m
            zr = persist.tile([S, NF], f32, tag="zr")
            V.tensor_tensor(zr[:], iof_t[:], oscan[:], Alu.subtract)

            # r = zr mod m  via  r = zr - m*round(zr/m), fixed up to [0, m).
            recip = persist.tile([S, 1], f32, tag="recip")
            V.reciprocal(recip[:], m_col)
            nm_col = persist.tile([S, 1], f32, tag="nm_col")
            V.tensor_scalar(nm_col[:], m_col, -1.0, None, Alu.mult)
            q_f = persist.tile([S, NF], f32, tag="q_f")
            V.tensor_scalar(q_f[:], zr[:], recip[:], None, Alu.mult)
            q_i = persist.tile([S, NF], dt.int32, tag="q_i")
            V.tensor_copy(q_i[:], q_f[:])
            V.tensor_copy(q_f[:], q_i[:])
            r_t = persist.tile([S, NF], f32, tag="r_t")
            V.scalar_tensor_tensor(r_t[:], q_f[:], nm_col, zr[:],
                                   Alu.mult, Alu.add)
            fix = persist.tile([S, NF], f32, tag="fix")
            V.tensor_scalar(fix[:], r_t[:], m_col, None, Alu.is_ge)
            V.scalar_tensor_tensor(r_t[:], fix[:], nm_col, r_t[:],
                                   Alu.mult, Alu.add)
            V.tensor_scalar(fix[:], r_t[:], 0.0, None, Alu.is_lt)
            V.scalar_tensor_tensor(r_t[:], fix[:], m_col, r_t[:],
                                   Alu.mult, Alu.add)

            # rho = mask ? oscan-1 : -1
            t1 = persist.tile([S, NF], f32, tag="t1")
            V.scalar_tensor_tensor(t1[:], oscan[:], -1.0, mask[:],
                                   Alu.add, Alu.mult)
            V.scalar_tensor_tensor(t1[:], mask[:], -1.0, t1[:],
                                   Alu.add, Alu.add)
            # ridx2 = mask ? rho : r    (source compact row per pixel)
            u_t = persist.tile([S, NF], f32, tag="u_t")
            V.tensor_tensor(u_t[:], t1[:], r_t[:], Alu.subtract)
            V.tensor_tensor(u_t[:], u_t[:], mask[:], Alu.mult)
            V.tensor_tensor(u_t[:], u_t[:], r_t[:], Alu.add)
            # scatter ridx2+1 into padded positions (0 at border/untouched)
            V.tensor_scalar(u_t[:], u_t[:], 1.0, None, Alu.add)
            r2i = persist.tile([S, NF], dt.int16, tag="r2i")
            V.tensor_copy(r2i[:], u_t[:])
            rp_i = persist.tile([S, JW], dt.int16, tag="rp_i")
            GP.local_scatter(rp_i[:], r2i[:], padf_t[:],
                             channels=S, num_elems=JW, num_idxs=NF)
            rp_bf = persist.tile([S, JW], dt.bfloat16, tag="rp_bf")
            V.tensor_copy(rp_bf[:], rp_i[:])

            # rho -> bf16, pad tail with -1, round-trip to DRAM, come back
            # pixel-major via 6 xbar transpose DMAs
            rho_bf = persist.tile([S, NK * 128], dt.bfloat16, tag="rho_bf")
            V.tensor_copy(rho_bf[:, 0:NF], t1[:])
            V.memset(rho_bf[:, NF:NK * 128], -1.0)
            wr_rho = nc.sync.dma_start(rho_d, rho_bf[:])
            rho_pm = persist.tile([128, NK, S], dt.bfloat16, tag="rho_pm")
            for k in range(NK):
                rd = nc.sync.dma_start(
                    rho_pm[:, k, :], rho_d[:, 128 * k:128 * (k + 1)],
                    transpose=True)
                add_dep_helper(rd.ins, wr_rho.ins,
                               reason="rho DRAM round-trip")

            # ---------------- per-sample pipeline
            NH = ((0, 14), (14, 13))      # output row halves: start row, nrows
            prev = None                   # (ut_sb, Pt) of even sample
            hyp4 = rp8 = yb4 = None
            for s in range(S):
                # batched input DMAs (one HWDGE op per 4/8 samples)
                if s % 4 == 0:
                    hyp4 = hpool.tile([128, 4, NK * 256], dt.bfloat16,
                                      tag="hyp")
                    nc.sync.dma_start(
                        hyp4[:], hyp[s:s + 4].rearrange("i p c -> p i c"))
                if s % 8 == 0:
                    rp8 = rppool.tile([1, 8 * JW], dt.bfloat16, tag="rp8")
                    nc.sync.dma_start(
                        rp8[:].rearrange("o (i j) -> o i j", j=JW),
                        rp_bf[s:s + 8, :])
                hyp_t = hyp4[:, s % 4]

                # Q one-hots for the whole pair in one DVE op:
                # Q2[p, s', k, r] = (rho[s+s'][k*128+p] == r)
                if s % 2 == 0:
                    Qt2 = qpool.tile([128, 2, NK, 128], dt.bfloat16, tag="Q")
                    rho_b = (rho_pm[:, :, s:s + 2]
                             .rearrange("p k s -> p s k")
                             .broadcast_to([128, 2, NK, 128]))
                    V.tensor_tensor(
                        Qt2[:],
                        iR_t[:].rearrange("p (a k r) -> p a k r",
                                          a=2, k=NK),
                        rho_b, Alu.is_equal)
                Qt = Qt2[:, s % 2]

                # hc[c, r] accumulated over pixel chunks (bank-padded psum)
                ps_hc = psA.tile([128, 2, 256], f32, tag="hc")
                nhk = 1 if "hc1" in mut else NK
                for cc in range(2):
                    for k in range(nhk):
                        nc.tensor.matmul(
                            ps_hc[:, cc, 0:128],
                            hyp_t[:, k * 256 + cc * 128:
                                  k * 256 + cc * 128 + 128],
                            Qt[:, k, :],
                            start=(k == 0), stop=(k == nhk - 1))
                hc_sb = hcpool.tile([128, 2, 128], dt.bfloat16, tag="hc_sb")
                A.activation(hc_sb[:], ps_hc[:, :, 0:128],
                             mybir.ActivationFunctionType.Identity)

                # UT[r, (t,o)] = sum_c hc[c,r] * wr[c,(t,o)]
                ps_u = psU.tile([128, 2, 512], f32, tag="u")
                for cc in range(2):
                    nc.tensor.matmul(ps_u[:, 0, 0:288], hc_sb[:, cc, :],
                                     wr_t[:, cc * 576:cc * 576 + 288],
                                     start=(cc == 0), stop=(cc == 1))
                    nc.tensor.matmul(ps_u[:, 1, 0:288], hc_sb[:, cc, :],
                                     wr_t[:, cc * 576 + 288:cc * 576 + 576],
                                     start=(cc == 0), stop=(cc == 1))
                ut_sb = utpool.tile([128, 576], dt.bfloat16, tag="ut")
                A.activation(ut_sb[:].rearrange("p (a b) -> p a b", b=288),
                             ps_u[:, :, 0:288],
                             mybir.ActivationFunctionType.Identity)

                # P one-hot: P[r, j] = (ridx2[j] == r): a ones-matmul
                # broadcasts the sample's index row (on partition 0 of the
                # batched rp8 tile) across partitions into a bank-aligned
                # PSUM pair, one is_equal one-hots it.
                j0 = (s % 8) * JW
                ps_p = psP.tile([128, 2, 512], f32, tag="p")
                nc.tensor.matmul(ps_p[:, 0, 0:421], ones_t[:],
                                 rp8[:, j0:j0 + 421], start=True, stop=True)
                nc.tensor.matmul(ps_p[:, 1, 0:421], ones_t[:],
                                 rp8[:, j0 + 421:j0 + 842],
                                 start=True, stop=True)
                Pt = ppool.tile([128, JW], dt.bfloat16, tag="P")
                V.tensor_scalar(Pt[:, 0:842].rearrange("p (a j) -> p a j",
                                                       j=421),
                                ps_p[:, :, 0:421], iC1_t[:], None,
                                Alu.is_equal)

                if s % 2 == 0:
                    prev = (ut_sb, Pt)
                    continue

                # conv for the pair (s-1, s): column-tiled matmuls
                utA, PA = prev
                utB, PB = ut_sb, Pt
                gA = PA[:, 0:NPAD].rearrange("p (i j) -> p i j", j=PP)
                gB = PB[:, 0:NPAD].rearrange("p (i j) -> p i j", j=PP)
                if s % 8 == 1:
                    yb4 = ybpool.tile([128, 4, N], f32, tag="yb4")
                ybuf = yb4[:, (s // 2) % 4]
                ntap = 1 if "conv1" in mut else 9
                for nh, (r0, nr) in enumerate(NH):
                    ps_yb = psY.tile([128, 512], f32, tag=f"y{nh}")
                    ps_y = ps_yb[:, 0:nr * P]
                    # interleaved A/B matmuls run concurrently on PE column
                    # halves; "seqconv" (sim-only) orders them group-contiguous
                    # for CoreSim's coarser psum-group model.
                    order = ([(t, h) for t in range(ntap) for h in range(2)]
                             if "seqconv" not in mut else
                             [(t, h) for h in range(2) for t in range(ntap)])
                    for t, h in order:
                        dy, dx = t // 3, t % 3
                        ut_h, g_h = (utA, gA) if h == 0 else (utB, gB)
                        nc.tensor.matmul(
                            ps_y[64 * h:64 * h + 64, :],
                            ut_h[:, t * 64:(t + 1) * 64],
                            g_h[:, r0 + dy:r0 + dy + nr, dx:dx + P],
                            start=(t == 0), stop=(t == ntap - 1),
                            skip_group_check=True)
                    A.activation(ybuf[:, r0 * P:(r0 + nr) * P], ps_y[:],
                                 mybir.ActivationFunctionType.Identity,
                                 bias=bias_t[:], scale=1.0)
                if s % 8 == 7:
                    nc.scalar.dma_start(
                        y[s - 7:s + 1].rearrange("(g a) o n -> a o g n", a=2),
                        yb4[:])

        if loop_n > 1:
            with tc.For_i(0, loop_n, 1):
                body()
        else:
            body()

    nc.compile()
    return nc


_NC_CACHE = {}


def _get_nc(loop_n=1):
    key = (loop_n, os.environ.get("KERNEL_MUT", ""))
    if key not in _NC_CACHE:
        _NC_CACHE[key] = build_nc(loop_n)
    return _NC_CACHE[key]


# ---------------------------------------------------------------- entry point

def kernel(x, W, b, rand_idx):
    from concourse.bass_utils import run_bass_kernel_spmd

    x = np.asarray(x, np.float32)
    W = np.asarray(W, np.float32)
    b = np.asarray(b, np.float32)
    rand_idx = np.asarray(rand_idx)

    in_maps = _host_stage(x, W, b, rand_idx)
    nc = _get_nc()
    res = run_bass_kernel_spmd(nc, in_maps, list(range(N_CORES)))
    out = np.concatenate([res.results[c]["y"] for c in range(N_CORES)], axis=0)
    return out.reshape(B, OUTC, P, P).astype(np.float32)


# revision 41
# speedup vs baseline: 1.1248x; 1.1248x over previous
"""Trainium2 Bass kernel for nn_DeformConv2D_EPF (scatter_memory).

Contract: kernel(**inputs) takes the FULL unsharded inputs
  x [512, 202, 27, 27] f32, W [64, 200, 3, 3] f32, b [64] f32,
  rand_idx [512, 58] int32
and returns the FULL output y [512, 64, 27, 27] f32.

Strategy (pure data parallel, batch 512 -> 8 cores x 64 samples), all
data-dependent work on device.

Key idea: per sample, every pixel of the deformed conv operand is one of
the <=128 "compact" rows (the central-superpixel pixels), i.e. the
operand has rank <= 128.  The whole scatter/gather is therefore
expressed as one-hot matmuls on the PE instead of SWDGE dma_gather
descriptor storms (the previous bottleneck at ~10us/sample):

  hc[c,r]    = sum_n hyper[n,c] * Q[n,r]     Q[n,r] = (rho[n] == r)
  UT[r,t,o]  = sum_c hc[c,r] * W[o,c,t]      per-tap weighted compact rows
  y[o,n]     = sum_t sum_r UT[r,t,o] * P[r, pad(n)+off(t)]
               P[r,j] = (ridx2[j] == r)      3x3 conv, contraction over r

Q and P are one-hots built by DVE is_equal against iota constants:
  - rho (compact rank per pixel, -1 if none) is computed in phase A
    ([samples, pixels] layout), round-trips through DRAM and comes back
    pixel-major via 6 xbar transpose DMAs -> per-(chunk,sample) scalar
    columns for Q.
  - ridx2 (source compact row per padded position, via zrank mod m) is
    scattered into padded layout by GPSIMD local_scatter, then
    partition-replicated per sample by one SBUF->SBUF broadcast DMA.

Conv matmuls are column-tiled in sample pairs: sample A's 18 taps write
PSUM partitions 0:64 (PE columns 0:63), sample B's write 64:128 via
tile_position (0,64), so two samples stream the 128-wide PE array
concurrently.  PE work/sample ~7k cycles; everything else overlaps.
"""

import os
from contextlib import ExitStack

import ml_dtypes
import numpy as np

B, C, P = 512, 202, 27
CH, N = 200, P * P            # 200, 729
PP = 29                       # padded side
NPAD = PP * PP                # 841
JW = 896                      # padded-position row width (841 padded)
NF = 736                      # pixel count padded for phase A
OUTC = 64
S = int(os.environ.get("KERNEL_S", "64"))   # samples per core (env: sim tests)
N_CORES = 8
CENTER = (P // 2) * P + P // 2  # 364
NK = 6                        # pixel chunks of 128 (768 padded)

_BF16 = ml_dtypes.bfloat16


def _stage_core(x, W, b, rand_idx):
    """Input map for one core's sample slice. Layout/dtype reformat only."""
    ns = x.shape[0]
    hyper = np.ascontiguousarray(
        x[:, :CH].reshape(ns, CH, N).transpose(0, 2, 1))     # [ns, N, CH]
    hyp = np.zeros((ns, NK * 128, 256), dtype=_BF16)
    hyp[:, :N, :CH] = hyper.astype(_BF16)
    # on-chip layout: partition = n%128, free = (k, c)
    hyp = np.ascontiguousarray(
        hyp.reshape(ns, NK, 128, 256).transpose(0, 2, 1, 3)
        .reshape(ns, 128, NK * 256))

    spr = np.zeros((ns, NF), np.float32)
    spr[:, :N] = x[:, CH].reshape(ns, N)

    rnd = np.full((ns, 64), -1, np.int16)
    rnd[:, :58] = rand_idx.astype(np.int16)

    wr = np.zeros((128, 1152), dtype=_BF16)
    Wt = W.transpose(1, 2, 3, 0).reshape(CH, 9, OUTC)         # [c, t, o]
    wr[:, :576] = Wt[:128].reshape(128, 576).astype(_BF16)
    wr[: CH - 128, 576:] = Wt[128:].reshape(CH - 128, 576).astype(_BF16)

    bias2 = np.tile(b.astype(np.float32), 2).reshape(128, 1)

    iota_f = np.broadcast_to(np.arange(NF, dtype=np.float32), (ns, NF)).copy()
    iota1_i = np.broadcast_to(
        (np.arange(NF) + 1).astype(np.int16), (ns, NF)).copy()
    n = np.arange(N)
    flatpad = (n // P + 1) * PP + (n % P + 1)
    padf0 = np.full(NF, -1, np.int16)
    padf0[:N] = flatpad
    padf = np.broadcast_to(padf0, (ns, NF)).copy()
    # iota along r, tiled for a whole (pair, chunk) Q build in one DVE op
    iotaR = np.broadcast_to(
        np.arange(128, dtype=np.float32), (128, 2 * NK * 128)
        if False else (128, 2, NK, 128)).reshape(128, 2 * NK * 128)
    iotaR = np.ascontiguousarray(iotaR).astype(_BF16)
    iotaC1 = (np.arange(128, dtype=np.float32) + 1).reshape(128, 1)
    ones = np.ones((1, 128), dtype=_BF16)

    return {
        "hyp": hyp, "spr": spr, "rnd": rnd, "wr": wr, "bias2": bias2,
        "c_iota_f": iota_f, "c_iota1": iota1_i, "c_padf": padf,
        "c_iotaR": iotaR, "c_iotaC1": iotaC1, "c_ones": ones,
    }


def _host_stage(x, W, b, rand_idx):
    in_maps = []
    for c in range(N_CORES):
        sl = slice(c * S, (c + 1) * S)
        in_maps.append(_stage_core(x[sl], W, b, rand_idx[sl]))
    return in_maps


# ---------------------------------------------------------------- bass build

def build_nc(loop_n=1, mut=None):
    mut = mut or os.environ.get("KERNEL_MUT", "")
    import concourse.bass as bass
    import concourse.tile as tile
    from concourse import bacc, mybir, library_config
    from concourse.tile_rust import add_dep_helper

    dt = mybir.dt
    Alu = mybir.AluOpType

    nc = bacc.Bacc("TRN2", target_bir_lowering=False, debug=False,
                   num_devices=N_CORES, num_swdge_queues=1)

    hyp = nc.dram_tensor("hyp", [S, 128, NK * 256], dt.bfloat16,
                         kind="ExternalInput").ap()
    spr = nc.dram_tensor("spr", [S, NF], dt.float32, kind="ExternalInput").ap()
    rnd = nc.dram_tensor("rnd", [S, 64], dt.int16, kind="ExternalInput").ap()
    wr = nc.dram_tensor("wr", [128, 1152], dt.bfloat16,
                        kind="ExternalInput").ap()
    bias2 = nc.dram_tensor("bias2", [128, 1], dt.float32,
                           kind="ExternalInput").ap()
    c_iota_f = nc.dram_tensor("c_iota_f", [S, NF], dt.float32,
                              kind="ExternalInput").ap()
    c_iota1 = nc.dram_tensor("c_iota1", [S, NF], dt.int16,
                             kind="ExternalInput").ap()
    c_padf = nc.dram_tensor("c_padf", [S, NF], dt.int16,
                            kind="ExternalInput").ap()
    c_iotaR = nc.dram_tensor("c_iotaR", [128, 2 * NK * 128], dt.bfloat16,
                             kind="ExternalInput").ap()
    c_iotaC1 = nc.dram_tensor("c_iotaC1", [128, 1], dt.float32,
                              kind="ExternalInput").ap()
    c_ones = nc.dram_tensor("c_ones", [1, 128], dt.bfloat16,
                            kind="ExternalInput").ap()
    y = nc.dram_tensor("y", [S, OUTC, N], dt.float32,
                       kind="ExternalOutput").ap()
    rho_d = nc.dram_tensor("rho_d", [S, NK * 128], dt.bfloat16,
                           kind="Internal").ap()

    with tile.TileContext(nc) as tc, ExitStack() as ctx:
        persist = ctx.enter_context(tc.tile_pool(name="persist", bufs=1))
        hpool = ctx.enter_context(tc.tile_pool(name="hpool", bufs=4))
        rppool = ctx.enter_context(tc.tile_pool(name="rppool", bufs=2))
        qpool = ctx.enter_context(tc.tile_pool(name="qpool", bufs=2))
        hcpool = ctx.enter_context(tc.tile_pool(name="hcpool", bufs=2))
        utpool = ctx.enter_context(tc.tile_pool(name="utpool", bufs=3))
        ppool = ctx.enter_context(tc.tile_pool(name="ppool", bufs=3))
        ybpool = ctx.enter_context(tc.tile_pool(name="ybpool", bufs=2))
        psA = ctx.enter_context(tc.tile_pool(name="psA", bufs=2, space="PSUM"))
        psU = ctx.enter_context(tc.tile_pool(name="psU", bufs=1, space="PSUM"))
        psP = ctx.enter_context(tc.tile_pool(name="psP", bufs=1, space="PSUM"))
        psY = ctx.enter_context(tc.tile_pool(name="psY", bufs=1, space="PSUM"))

        V = nc.vector
        A = nc.scalar
        GP = nc.gpsimd
        f32 = dt.float32

        def body():
            # ---------------- persistent loads
            def load(name, src, shape, dty):
                t = persist.tile(shape, dty, tag=name)
                nc.sync.dma_start(t[:], src)
                return t

            sp_t = load("sp", spr, [S, NF], f32)
            rnd_t = load("rnd", rnd, [S, 64], dt.int16)
            wr_t = load("wr", wr, [128, 1152], dt.bfloat16)
            bias_t = load("bias2", bias2, [128, 1], f32)
            iof_t = load("iof", c_iota_f, [S, NF], f32)
            io1_t = load("io1", c_iota1, [S, NF], dt.int16)
            padf_t = load("padf", c_padf, [S, NF], dt.int16)
            iR_t = load("iR", c_iotaR, [128, 2 * NK * 128], dt.bfloat16)
            iC1_t = load("iC1", c_iotaC1, [128, 1], f32)
            ones_t = load("ones", c_ones, [1, 128], dt.bfloat16)

            GP.load_library(library_config.local_scatter)

            # ---------------- phase A: index math (samples on partitions)
            central = persist.tile([S, 1], f32, tag="central")
            V.tensor_copy(central[:], sp_t[:, CENTER:CENTER + 1])

            mask = persist.tile([S, NF], f32, tag="mask")
            V.tensor_scalar(mask[:], sp_t[:], central[:], None, Alu.is_equal)
            # rand one-hot via per-partition scatter of positive data
            rsc = persist.tile([S, NF], dt.int16, tag="rsc")
            rscf = persist.tile([S, NF], f32, tag="rscf")
            GP.local_scatter(rsc[:], io1_t[:, 0:64], rnd_t[:],
                             channels=S, num_elems=NF, num_idxs=64)
            V.tensor_copy(rscf[:], rsc[:])
            V.tensor_scalar(rscf[:], rscf[:], 0.0, None, Alu.is_gt)
            V.tensor_tensor(mask[:], mask[:], rscf[:], Alu.max)

            oscan = persist.tile([S, NF], f32, tag="oscan")
            V.tensor_tensor_scan(oscan[:], mask[:], mask[:], 0.0,
                                 Alu.add, Alu.bypass)
            m_col = oscan[:, NF - 1:NF]                 # ones count m
            zr = persist.tile([S, NF], f32, tag="zr")
            V.tensor_tensor(zr[:], iof_t[:], oscan[:], Alu.subtract)

            # r = zr mod m  via  r = zr - m*round(zr/m), fixed up to [0, m).
            recip = persist.tile([S, 1], f32, tag="recip")
            V.reciprocal(recip[:], m_col)
            nm_col = persist.tile([S, 1], f32, tag="nm_col")
            V.tensor_scalar(nm_col[:], m_col, -1.0, None, Alu.mult)
            q_f = persist.tile([S, NF], f32, tag="q_f")
            V.tensor_scalar(q_f[:], zr[:], recip[:], None, Alu.mult)
            q_i = persist.tile([S, NF], dt.int32, tag="q_i")
            V.tensor_copy(q_i[:], q_f[:])
            V.tensor_copy(q_f[:], q_i[:])
            r_t = persist.tile([S, NF], f32, tag="r_t")
            V.scalar_tensor_tensor(r_t[:], q_f[:], nm_col, zr[:],
                                   Alu.mult, Alu.add)
            fix = persist.tile([S, NF], f32, tag="fix")
            V.tensor_scalar(fix[:], r_t[:], m_col, None, Alu.is_ge)
            V.scalar_tensor_tensor(r_t[:], fix[:], nm_col, r_t[:],
                                   Alu.mult, Alu.add)
            V.tensor_scalar(fix[:], r_t[:], 0.0, None, Alu.is_lt)
            V.scalar_tensor_tensor(r_t[:], fix[:], m_col, r_t[:],
                                   Alu.mult, Alu.add)

            # rho = mask ? oscan-1 : -1
            t1 = persist.tile([S, NF], f32, tag="t1")
            V.scalar_tensor_tensor(t1[:], oscan[:], -1.0, mask[:],
                                   Alu.add, Alu.mult)
            V.scalar_tensor_tensor(t1[:], mask[:], -1.0, t1[:],
                                   Alu.add, Alu.add)
            # ridx2 = mask ? rho : r    (source compact row per pixel)
            u_t = persist.tile([S, NF], f32, tag="u_t")
            V.tensor_tensor(u_t[:], t1[:], r_t[:], Alu.subtract)
            V.tensor_tensor(u_t[:], u_t[:], mask[:], Alu.mult)
            V.tensor_tensor(u_t[:], u_t[:], r_t[:], Alu.add)
            # scatter ridx2+1 into padded positions (0 at border/untouched)
            V.tensor_scalar(u_t[:], u_t[:], 1.0, None, Alu.add)
            r2i = persist.tile([S, NF], dt.int16, tag="r2i")
            V.tensor_copy(r2i[:], u_t[:])
            rp_i = persist.tile([S, JW], dt.int16, tag="rp_i")
            GP.local_scatter(rp_i[:], r2i[:], padf_t[:],
                             channels=S, num_elems=JW, num_idxs=NF)
            rp_bf = persist.tile([S, JW], dt.bfloat16, tag="rp_bf")
            V.tensor_copy(rp_bf[:], rp_i[:])

            # rho -> bf16, pad tail with -1, round-trip to DRAM, come back
            # pixel-major via 6 xbar transpose DMAs
            rho_bf = persist.tile([S, NK * 128], dt.bfloat16, tag="rho_bf")
            V.tensor_copy(rho_bf[:, 0:NF], t1[:])
            V.memset(rho_bf[:, NF:NK * 128], -1.0)
            wr_rho = nc.sync.dma_start(rho_d, rho_bf[:])
            rho_pm = persist.tile([128, NK, S], dt.bfloat16, tag="rho_pm")
            for k in range(NK):
                rd = nc.sync.dma_start(
                    rho_pm[:, k, :], rho_d[:, 128 * k:128 * (k + 1)],
                    transpose=True)
                add_dep_helper(rd.ins, wr_rho.ins,
                               reason="rho DRAM round-trip")

            # ---------------- per-sample pipeline
            NH = ((0, 14), (14, 13))      # output row halves: start row, nrows
            prev = None                   # (ut_sb, Pt) of even sample
            hyp4 = rp8 = yb4 = None
            for s in range(S):
                hyp_t = hpool.tile([128, NK * 256], dt.bfloat16, tag="hyp")
                nc.sync.dma_start(hyp_t[:], hyp[s])
                # index rows for the P broadcast, batched 8 samples per DMA
                # (their source is ready at phase A, so the coarse dep is
                # harmless and saves 56 HWDGE ops)
                if s % 8 == 0:
                    rp8 = rppool.tile([1, 8 * JW], dt.bfloat16, tag="rp8")
                    nc.sync.dma_start(
                        rp8[:].rearrange("o (i j) -> o i j", j=JW),
                        rp_bf[s:s + 8, :])

                # Q one-hots for the whole pair in one DVE op:
                # Q2[p, s', k, r] = (rho[s+s'][k*128+p] == r)
                if s % 2 == 0:
                    Qt2 = qpool.tile([128, 2, NK, 128], dt.bfloat16, tag="Q")
                    rho_b = (rho_pm[:, :, s:s + 2]
                             .rearrange("p k s -> p s k")
                             .broadcast_to([128, 2, NK, 128]))
                    V.tensor_tensor(
                        Qt2[:],
                        iR_t[:].rearrange("p (a k r) -> p a k r",
                                          a=2, k=NK),
                        rho_b, Alu.is_equal)
                Qt = Qt2[:, s % 2]

                # hc[c, r] accumulated over pixel chunks (bank-padded psum)
                ps_hc = psA.tile([128, 2, 256], f32, tag="hc")
                nhk = 1 if "hc1" in mut else NK
                for cc in range(2):
                    for k in range(nhk):
                        nc.tensor.matmul(
                            ps_hc[:, cc, 0:128],
                            hyp_t[:, k * 256 + cc * 128:
                                  k * 256 + cc * 128 + 128],
                            Qt[:, k, :],
                            start=(k == 0), stop=(k == nhk - 1))
                hc_sb = hcpool.tile([128, 2, 128], dt.bfloat16, tag="hc_sb")
                A.activation(hc_sb[:], ps_hc[:, :, 0:128],
                             mybir.ActivationFunctionType.Identity)

                # UT[r, (t,o)] = sum_c hc[c,r] * wr[c,(t,o)]
                ps_u = psU.tile([128, 2, 512], f32, tag="u")
                for cc in range(2):
                    nc.tensor.matmul(ps_u[:, 0, 0:288], hc_sb[:, cc, :],
                                     wr_t[:, cc * 576:cc * 576 + 288],
                                     start=(cc == 0), stop=(cc == 1))
                    nc.tensor.matmul(ps_u[:, 1, 0:288], hc_sb[:, cc, :],
                                     wr_t[:, cc * 576 + 288:cc * 576 + 576],
                                     start=(cc == 0), stop=(cc == 1))
                ut_sb = utpool.tile([128, 576], dt.bfloat16, tag="ut")
                A.activation(ut_sb[:].rearrange("p (a b) -> p a b", b=288),
                             ps_u[:, :, 0:288],
                             mybir.ActivationFunctionType.Identity)

                # P one-hot: P[r, j] = (ridx2[j] == r): a ones-matmul
                # broadcasts the sample's index row (on partition 0 of the
                # batched rp8 tile) across partitions into a bank-aligned
                # PSUM pair, one is_equal one-hots it.
                j0 = (s % 8) * JW
                ps_p = psP.tile([128, 2, 512], f32, tag="p")
                nc.tensor.matmul(ps_p[:, 0, 0:421], ones_t[:],
                                 rp8[:, j0:j0 + 421], start=True, stop=True)
                nc.tensor.matmul(ps_p[:, 1, 0:421], ones_t[:],
                                 rp8[:, j0 + 421:j0 + 842],
                                 start=True, stop=True)
                Pt = ppool.tile([128, JW], dt.bfloat16, tag="P")
                V.tensor_scalar(Pt[:, 0:842].rearrange("p (a j) -> p a j",
                                                       j=421),
                                ps_p[:, :, 0:421], iC1_t[:], None,
                                Alu.is_equal)

                if s % 2 == 0:
                    prev = (ut_sb, Pt)
                    continue

                # conv for the pair (s-1, s): column-tiled matmuls
                utA, PA = prev
                utB, PB = ut_sb, Pt
                gA = PA[:, 0:NPAD].rearrange("p (i j) -> p i j", j=PP)
                gB = PB[:, 0:NPAD].rearrange("p (i j) -> p i j", j=PP)
                ybuf = ybpool.tile([128, N], f32, tag="ybuf")
                ntap = 1 if "conv1" in mut else 9
                for nh, (r0, nr) in enumerate(NH):
                    ps_yb = psY.tile([128, 512], f32, tag=f"y{nh}")
                    ps_y = ps_yb[:, 0:nr * P]
                    # interleaved A/B matmuls run concurrently on PE column
                    # halves; "seqconv" (sim-only) orders them group-contiguous
                    # for CoreSim's coarser psum-group model.
                    order = ([(t, h) for t in range(ntap) for h in range(2)]
                             if "seqconv" not in mut else
                             [(t, h) for h in range(2) for t in range(ntap)])
                    for t, h in order:
                        dy, dx = t // 3, t % 3
                        ut_h, g_h = (utA, gA) if h == 0 else (utB, gB)
                        nc.tensor.matmul(
                            ps_y[64 * h:64 * h + 64, :],
                            ut_h[:, t * 64:(t + 1) * 64],
                            g_h[:, r0 + dy:r0 + dy + nr, dx:dx + P],
                            start=(t == 0), stop=(t == ntap - 1),
                            skip_group_check=True)
                    A.activation(ybuf[:, r0 * P:(r0 + nr) * P], ps_y[:],
                                 mybir.ActivationFunctionType.Identity,
                                 bias=bias_t[:], scale=1.0)
                nc.scalar.dma_start(
                    y[s - 1:s + 1].rearrange("s o n -> (s o) n"), ybuf[:])

        if loop_n > 1:
            with tc.For_i(0, loop_n, 1):
                body()
        else:
            body()

    nc.compile()
    return nc


_NC_CACHE = {}


def _get_nc(loop_n=1):
    key = (loop_n, os.environ.get("KERNEL_MUT", ""))
    if key not in _NC_CACHE:
        _NC_CACHE[key] = build_nc(loop_n)
    return _NC_CACHE[key]


# ---------------------------------------------------------------- entry point

def kernel(x, W, b, rand_idx):
    from concourse.bass_utils import run_bass_kernel_spmd

    x = np.asarray(x, np.float32)
    W = np.asarray(W, np.float32)
    b = np.asarray(b, np.float32)
    rand_idx = np.asarray(rand_idx)

    in_maps = _host_stage(x, W, b, rand_idx)
    nc = _get_nc()
    res = run_bass_kernel_spmd(nc, in_maps, list(range(N_CORES)))
    out = np.concatenate([res.results[c]["y"] for c in range(N_CORES)], axis=0)
    return out.reshape(B, OUTC, P, P).astype(np.float32)
